# revision 18
# baseline (speedup 1.0000x reference)
"""Causal multi-head attention on 8 Trainium2 NeuronCores.

Sharding: data-parallel over batch (B=2) x tensor-parallel over heads
(16 heads -> 4 groups of 4). Core c handles batch c//4, head group c%4.
Each core computes q/k/v projections for its 4 heads, causal flash
attention, and a partial output projection (row slice of Wo); the host
sums the 4 partials per batch element.

Matmuls run in bf16 (fp32 PSUM accumulation): the PE streams 2-byte
moving operands at 1 cycle/row vs 2 for 4-byte. Inputs are cast to bf16
on the host. The softmax row-sum is fused into the o^T = [v|1s]^T P^T
matmul via an appended ones column; normalization (broadcast rowsum via
K=1 matmul reading partition 64, fast-approx reciprocal, divide) stays
in fp32.

To keep the PE dense (and its HAM clock-gate at 8/8), both big phases
are software-pipelined at the instruction-emission level:
  phase 1: x-transpose groups for s-chunk sc interleave with the q/k/v
           projection matmuls of s-chunk sc-1.
  phase 2: per (q-chunk, head-pair) only the S^T = k q^T matmuls and the
           masked exp are emitted in the main loop; AV matmuls,
           normalization epilogue, and the previous q-chunk's output
           projection drain from a work queue between them.
"""

import numpy as np
import ml_dtypes

import concourse.bacc as bacc
import concourse.bass as bass
import concourse.tile as tile
from concourse import bass_utils, mybir
from concourse.masks import make_identity

B, S, D, H = 2, 2048, 1024, 16
DK = 64
NH = 4                 # heads per core
E = NH * DK            # 256: per-core head-dim slice
SCALE = 1.0 / 8.0      # 1/sqrt(DK)
NEG = -30000.0

F32 = mybir.dt.float32
F32R = mybir.dt.float32r
BF16 = mybir.dt.bfloat16

QC = 512               # q-chunk (columns per attention tile)
NQC = S // QC          # 4
NKB = S // 128         # 16 k-blocks


def _emit(tc, nc, x_d, wq_d, wk_d, wv_d, wo_d, yT_d):
    const = tc.alloc_tile_pool(name="const", bufs=1)
    perm = tc.alloc_tile_pool(name="perm", bufs=1)
    p01 = tc.alloc_tile_pool(name="p01", bufs=1)
    nat = tc.alloc_tile_pool(name="nat", bufs=4)

    # HAM warmup: ~16 back-to-back matmuls on zeros flip the PE clock gate
    # to 8/8 within ~3.4us, so the transposes/projections that follow run
    # at 2.4 GHz instead of 1.2.
    warm_rhs = const.tile([128, 512], BF16)
    nc.vector.memset(warm_rhs, 0.0)
    identity = const.tile([128, 128], BF16)
    make_identity(nc, identity)
    # ones row for the rowsum broadcast (row 64 used as lhsT)
    ones128 = const.tile([128, 64], F32R)
    ones_f32 = const.tile([128, 64], F32)
    nc.gpsimd.memset(ones_f32, 1.0)
    nc.vector.tensor_copy(ones128, ones_f32)
    # causal triangle mask for diagonal 128x128 blocks of S^T
    # (partition r = key index, free c = query index): keep 0 where r <= c,
    # else a large negative so exp() underflows to exactly 0.
    mask = const.tile([128, 128], F32)
    nc.gpsimd.memset(mask, 0.0)
    # out[r, c] = (c - r) >= 0 ? 0.0 : NEG
    nc.gpsimd.affine_select(
        out=mask,
        in_=mask,
        compare_op=mybir.AluOpType.is_ge,
        fill=NEG,
        base=0,
        pattern=[[1, 128]],
        channel_multiplier=-1,
    )

    woT = perm.tile([128, 2, D], BF16)   # woT[p, ec, o] = wo[o, ec*128+p]
    qT = perm.tile([128, 2, S], BF16)    # qT[p, ec, s] = q[s, ec*128+p]
    kT = perm.tile([128, 2, S], BF16)
    v_sb = perm.tile([128, NKB, NH, DK + 1], BF16)  # [.., 64] = ones column

    xT = p01.tile([128, 8, S], BF16)     # xT[p, dc, s] = x[s, dc*128+p]
    wqT = p01.tile([128, 8, E], BF16)    # wqT[p, dc, e] = wq[e, dc*128+p]
    wkT = p01.tile([128, 8, E], BF16)
    wvT = p01.tile([128, 8, E], BF16)

    ncopy = [0]

    def copy(dst, src):
        if ncopy[0] % 2 == 0:
            nc.vector.tensor_copy(dst, src)
        else:
            nc.scalar.copy(dst, src)
        ncopy[0] += 1

    with tc.tile_pool(name="ps01", bufs=1, space="PSUM") as ps01:
        warm_ps = ps01.tile([128, 512], F32, tag="warm", bufs=1)
        for _ in range(16):
            nc.tensor.matmul(warm_ps, lhsT=warm_rhs[:, 0:128], rhs=warm_rhs,
                             start=True, stop=True)
        warm_sink = const.tile([128, 4], F32)
        nc.vector.tensor_copy(warm_sink, warm_ps[:, 0:4])

        # ---- phase 0: weight transposes via the DMA xbar (scalar HWDGE
        # queue, parallel to the x loads on sync), off the PE entirely ----
        for w_d, wT in [(wq_d, wqT), (wk_d, wkT), (wv_d, wvT)]:
            for dc in range(8):
                nc.scalar.dma_start(
                    out=wT[:, dc, :],
                    in_=w_d[:, dc * 128:(dc + 1) * 128],
                    transpose=True,
                )
        for ec in range(2):
            nc.scalar.dma_start(
                out=woT[:, ec, :],
                in_=wo_d[:, ec * 128:(ec + 1) * 128],
                transpose=True,
            )

        # ones column of v (written once; strided 3D AP)
        ones_ap = bass.AP(
            tensor=v_sb.tensor,
            offset=v_sb.offset + DK,
            ap=[v_sb.ap[0], [NH * (DK + 1), NKB], [DK + 1, NH]],
        )
        src64 = bass.AP(
            tensor=ones_f32.tensor, offset=ones_f32.offset,
            ap=[ones_f32.ap[0], [4, NKB], [1, NH]],
        )
        nc.vector.tensor_copy(ones_ap, src64)

        # ---- phase 1: x transposes for chunk sc interleaved with the
        # projection matmuls of chunk sc-1 (keeps HAM warm: transpose-mode
        # does not count as PE activity) ----
        def make_proj_units(sc):
            units = []
            for w_t, outT in [(wqT, qT), (wkT, kT)]:
                for ec in range(2):
                    def u(w_t=w_t, outT=outT, ec=ec, sc=sc):
                        ps = ps01.tile([128, 512], F32, tag="proj", bufs=4,
                                       name="psp")
                        for dc in range(8):
                            nc.tensor.matmul(
                                ps,
                                lhsT=w_t[:, dc, ec * 128:(ec + 1) * 128],
                                rhs=xT[:, dc, sc * 512:(sc + 1) * 512],
                                start=(dc == 0),
                                stop=(dc == 7),
                            )
                        copy(outT[:, ec, sc * 512:(sc + 1) * 512], ps)
                    units.append(u)
            return units

        prev_units = []
        for sc in range(4):
            pu = iter(prev_units)
            for j4 in range(4):
                sblk = sc * 4 + j4
                x_nat = nat.tile([128, D], BF16, tag="xnat", bufs=4)
                nc.sync.dma_start(out=x_nat, in_=x_d[sblk * 128:(sblk + 1) * 128, :])
                for dcg in range(2):
                    ps = ps01.tile([128, 512], BF16, tag="tp", bufs=2)
                    for j in range(4):
                        dc = dcg * 4 + j
                        nc.tensor.transpose(
                            ps[:, j * 128:(j + 1) * 128],
                            x_nat[:, dc * 128:(dc + 1) * 128],
                            identity,
                        )
                    copy(xT[:, dcg * 4:(dcg + 1) * 4, sblk * 128:(sblk + 1) * 128], ps)
                    u = next(pu, None)
                    if u is not None:
                        u()
                    else:
                        # keep the HAM clock-gate warm: transposes do not
                        # count as PE activity
                        for _ in range(3):
                            nc.tensor.matmul(warm_ps, lhsT=warm_rhs[:, 0:128],
                                             rhs=warm_rhs, start=True, stop=True)
            for u in pu:
                u()
            prev_units = make_proj_units(sc)
        for u in prev_units:
            u()

    nat.release()

    work = tc.alloc_tile_pool(name="work", bufs=3)
    small = tc.alloc_tile_pool(name="small", bufs=2)

    # ---- phase 2/3: attention + output projection, work-queue pipelined ----
    with tc.tile_pool(name="psS", bufs=1, space="PSUM") as ps_S, \
         tc.tile_pool(name="psO", bufs=1, space="PSUM") as ps_o, \
         tc.tile_pool(name="psY", bufs=1, space="PSUM") as ps_y:

        workq = []  # (weight, closure) of ready-to-run background PE work

        def make_vproj(sblk):
            def u():
                ps = ps_y.tile([128, E], F32, tag="y", bufs=2, name="psv")
                for dc in range(8):
                    nc.tensor.matmul(
                        ps,
                        lhsT=xT[:, dc, sblk * 128:(sblk + 1) * 128],
                        rhs=wvT[:, dc, :],
                        start=(dc == 0),
                        stop=(dc == 7),
                    )
                # scatter 4 heads into [.., l, 0:64]
                sap = bass.AP(
                    tensor=ps.tensor, offset=ps.offset,
                    ap=[ps.ap[0], [DK, NH], [1, DK]],
                )
                nc.vector.tensor_copy(v_sb[:, sblk, :, 0:DK], sap)
            return u

        for sblk in range(NKB):
            workq.append((2, make_vproj(sblk)))

        def make_av(po_box, pts, kb, hp, kmax, qc):
            def av():
                if po_box[0] is None:
                    po_box[0] = (
                        ps_o.tile([DK + 1, QC], F32, tag="o", bufs=2, name="poA"),
                        ps_o.tile([DK + 1, QC], F32, tag="o", bufs=2, name="poB"),
                    )
                poA, poB = po_box[0]
                pT, cs = pts[kb]
                for hi, po in ((0, poA), (1, poB)):
                    nc.tensor.matmul(
                        po[:, cs:512],
                        lhsT=v_sb[:, kb, 2 * hp + hi, :],
                        rhs=pT[:, hi, cs:512],
                        start=(kb == 0),
                        stop=(kb == kmax - 1),
                    )
            return av

        def make_epi_copies(po_box, sb_box):
            def epi_c():
                poA, poB = po_box[0]
                # free the psum banks quickly with one copy per head
                oA_sb = small.tile([DK + 1, QC], F32R, tag="osb", bufs=4)
                oB_sb = small.tile([DK + 1, QC], F32R, tag="osb", bufs=4)
                nc.vector.tensor_copy(oA_sb, poA)
                nc.vector.tensor_copy(oB_sb, poB)
                sb_box[0] = (oA_sb, oB_sb)
            return epi_c

        def make_epilogue(sb_box, oT, hp):
            def epi():
                oA_sb, oB_sb = sb_box[0]
                for hi, o_sb in ((0, oA_sb), (1, oB_sb)):
                    # broadcast rowsum (row 64) to 64 partitions via K=1
                    # matmul reading partition 64 (row group 64)
                    ps_bc = ps_y.tile([64, QC], F32, tag="y", bufs=2, name="psbc")
                    nc.tensor.matmul(
                        ps_bc,
                        lhsT=ones128[64:65, :],
                        rhs=o_sb[DK:DK + 1, :],
                        start=True,
                        stop=True,
                    )
                    rec = small.tile([64, QC], F32, tag="rec", bufs=2)
                    nc.vector.reciprocal_approx_fast(rec, ps_bc)
                    if hi == 0:
                        nc.vector.tensor_mul(oT[0:64, hp, :], o_sb[0:DK, :], rec)
                    else:
                        tmpB = small.tile([64, QC], BF16, tag="tmpB", bufs=2)
                        nc.vector.tensor_mul(tmpB, o_sb[0:DK, :], rec)
                        # partition shift 0-63 -> 64-127 via sbuf->sbuf DMA
                        nc.sync.dma_start(out=oT[64:128, hp, :], in_=tmpB)
            return epi

        def make_out_proj(qc, oT):
            units = []
            for dc in range(8):
                def u(dc=dc, qc=qc, oT=oT):
                    psy = ps_y.tile([128, QC], F32, tag="y", bufs=2, name="psy")
                    for ec in range(2):
                        nc.tensor.matmul(
                            psy,
                            lhsT=woT[:, ec, dc * 128:(dc + 1) * 128],
                            rhs=oT[:, ec, :],
                            start=(ec == 0),
                            stop=(ec == 1),
                        )
                    y_sb = work.tile([128, QC], F32, tag="ysb", bufs=3)
                    nc.vector.tensor_copy(y_sb, psy)
                    nc.sync.dma_start(
                        out=yT_d[dc * 128:(dc + 1) * 128, qc * QC:(qc + 1) * QC],
                        in_=y_sb,
                    )
                units.append(u)
            return units

        pending_op = []
        for qc in range(NQC - 1, -1, -1):
            oT = work.tile([128, 2, QC], BF16, tag="oT", bufs=2)
            kmax = 4 * (qc + 1)
            for hp in range(2):
                pts = {}
                po_box = [None]
                sb_box = [None]
                for kb in range(kmax):
                    # S^T = k q^T, 2-head row-tiled pair, causally narrowed
                    cs = max(0, kb * 128 - qc * QC)
                    psS = ps_S.tile([128, 2, 512], F32, tag="S", bufs=2)
                    for hi in range(2):
                        nc.tensor.matmul(
                            psS[:, hi, cs:512],
                            lhsT=kT[hi * 64:(hi + 1) * 64, hp,
                                    kb * 128:(kb + 1) * 128],
                            rhs=qT[hi * 64:(hi + 1) * 64, hp,
                                   qc * QC + cs:(qc + 1) * QC],
                            start=True,
                            stop=True,
                        )
                    if kb >= 4 * qc:  # diagonal band: triangular mask
                        for hi in range(2):
                            nc.vector.tensor_add(
                                psS[:, hi, cs:cs + 128],
                                psS[:, hi, cs:cs + 128],
                                mask,
                            )
                    pT = work.tile([128, 2, 512], BF16, tag="pT", bufs=24)
                    pts[kb] = (pT, cs)
                    nc.scalar.activation(
                        pT[:, :, cs:512],
                        psS[:, :, cs:512],
                        mybir.ActivationFunctionType.Exp,
                        scale=SCALE,
                    )
                    budget = 2 if len(workq) > 12 else 1
                    while workq and budget > 0:
                        w, u = workq.pop(0)
                        u()
                        budget -= w
                # queue this phase's AV work + epilogue
                for kb in range(kmax):
                    workq.append((1, make_av(po_box, pts, kb, hp, kmax, qc)))
                workq.append((0, make_epi_copies(po_box, sb_box)))
                workq.append((1, make_epilogue(sb_box, oT, hp)))
                workq.extend(pending_op)
                pending_op = []
                if hp == 1:
                    pending_op = [(1, u) for u in make_out_proj(qc, oT)]
        for _, u in workq + pending_op:
            u()

    for p in [small, work, p01, perm, const]:
        p.release()


_CACHE = {}


def _build():
    if "nc" in _CACHE:
        return _CACHE["nc"]
    nc = bacc.Bacc("TRN2", target_bir_lowering=False, debug=False, num_devices=8)
    x_d = nc.dram_tensor("x", [S, D], BF16, kind="ExternalInput").ap()
    wq_d = nc.dram_tensor("wq", [E, D], BF16, kind="ExternalInput").ap()
    wk_d = nc.dram_tensor("wk", [E, D], BF16, kind="ExternalInput").ap()
    wv_d = nc.dram_tensor("wv", [E, D], BF16, kind="ExternalInput").ap()
    wo_d = nc.dram_tensor("wo", [D, E], BF16, kind="ExternalInput").ap()
    yT_d = nc.dram_tensor("yT", [D, S], F32, kind="ExternalOutput").ap()
    with tile.TileContext(nc) as tc:
        _emit(tc, nc, x_d, wq_d, wk_d, wv_d, wo_d, yT_d)
    nc.compile()
    _CACHE["nc"] = nc
    return nc


LAST_RESULT = None


def kernel(x, wq, wk, wv, wo):
    global LAST_RESULT
    nc = _build()
    bf = ml_dtypes.bfloat16
    x = np.asarray(x, dtype=np.float32)
    wq16 = np.asarray(wq, dtype=np.float32).astype(bf)
    wk16 = np.asarray(wk, dtype=np.float32).astype(bf)
    wv16 = np.asarray(wv, dtype=np.float32).astype(bf)
    wo16 = np.asarray(wo, dtype=np.float32).astype(bf)

    in_maps = []
    for c in range(8):
        b, g = c // 4, c % 4
        rows = slice(g * E, (g + 1) * E)
        in_maps.append({
            "x": np.ascontiguousarray(x[b].astype(bf)),
            "wq": np.ascontiguousarray(wq16[rows]),
            "wk": np.ascontiguousarray(wk16[rows]),
            "wv": np.ascontiguousarray(wv16[rows]),
            "wo": np.ascontiguousarray(wo16[:, rows]),
        })

    res = bass_utils.run_bass_kernel_spmd(nc, in_maps, core_ids=list(range(8)))
    LAST_RESULT = res

    y = np.empty((B, S, D), dtype=np.float32)
    for b in range(B):
        acc = res.results[4 * b]["yT"].copy()
        for g in range(1, 4):
            acc += res.results[4 * b + g]["yT"]
        y[b] = acc.T
    return y


# revision 19
# speedup vs baseline: 1.2950x; 1.2950x over previous
"""Causal multi-head attention on 8 Trainium2 NeuronCores.

Sharding: data-parallel over batch (B=2) x tensor-parallel over heads
(16 heads -> 4 groups of 4). Core c handles batch c//4, head group c%4.
Each core computes q/k/v projections for its 4 heads, causal flash
attention, and a partial output projection (row slice of Wo); the host
sums the 4 partials per batch element.

Matmuls run in bf16 (fp32 PSUM accumulation): the PE streams 2-byte
moving operands at 1 cycle/row vs 2 for 4-byte. Inputs are cast to bf16
on the host. The softmax row-sum is fused into the o^T = [v|1s]^T P^T
matmul via an appended ones column; normalization (broadcast rowsum via
K=1 matmul reading partition 64, fast-approx reciprocal, divide) stays
in fp32.

To keep the PE dense (and its HAM clock-gate at 8/8), both big phases
are software-pipelined at the instruction-emission level:
  phase 1: x-transpose groups for s-chunk sc interleave with the q/k/v
           projection matmuls of s-chunk sc-1.
  phase 2: per (q-chunk, head-pair) only the S^T = k q^T matmuls and the
           masked exp are emitted in the main loop; AV matmuls,
           normalization epilogue, and the previous q-chunk's output
           projection drain from a work queue between them.
"""

import numpy as np
import ml_dtypes

import concourse.bacc as bacc
import concourse.bass as bass
import concourse.tile as tile
from concourse import bass_utils, mybir
from concourse.masks import make_identity

B, S, D, H = 2, 2048, 1024, 16
DK = 64
NH = 4                 # heads per core
E = NH * DK            # 256: per-core head-dim slice
SCALE = 1.0 / 8.0      # 1/sqrt(DK)
NEG = -30000.0

F32 = mybir.dt.float32
F32R = mybir.dt.float32r
BF16 = mybir.dt.bfloat16

QC = 512               # q-chunk (columns per attention tile)
NQC = S // QC          # 4
NKB = S // 128         # 16 k-blocks


def _emit(tc, nc, x_d, wq_d, wk_d, wv_d, wo_d, yT_d):
    const = tc.alloc_tile_pool(name="const", bufs=1)
    perm = tc.alloc_tile_pool(name="perm", bufs=1)
    p01 = tc.alloc_tile_pool(name="p01", bufs=1)
    nat = tc.alloc_tile_pool(name="nat", bufs=4)

    # HAM warmup: ~16 back-to-back matmuls on zeros flip the PE clock gate
    # to 8/8 within ~3.4us, so the transposes/projections that follow run
    # at 2.4 GHz instead of 1.2.
    warm_rhs = const.tile([128, 512], BF16)
    nc.vector.memset(warm_rhs, 0.0)
    identity = const.tile([128, 128], BF16)
    make_identity(nc, identity)
    # ones row for the rowsum broadcast (row 64 used as lhsT)
    ones128 = const.tile([128, 64], F32R)
    ones_f32 = const.tile([128, 64], F32)
    nc.gpsimd.memset(ones_f32, 1.0)
    nc.vector.tensor_copy(ones128, ones_f32)
    # causal triangle mask for diagonal 128x128 blocks of S^T
    # (partition r = key index, free c = query index): keep 0 where r <= c,
    # else a large negative so exp() underflows to exactly 0.
    mask = const.tile([128, 128], F32)
    nc.gpsimd.memset(mask, 0.0)
    # out[r, c] = (c - r) >= 0 ? 0.0 : NEG
    nc.gpsimd.affine_select(
        out=mask,
        in_=mask,
        compare_op=mybir.AluOpType.is_ge,
        fill=NEG,
        base=0,
        pattern=[[1, 128]],
        channel_multiplier=-1,
    )

    woT = perm.tile([128, 2, D], BF16)   # woT[p, ec, o] = wo[o, ec*128+p]
    qT = perm.tile([128, 2, S], BF16)    # qT[p, ec, s] = q[s, ec*128+p]
    kT = perm.tile([128, 2, S], BF16)
    v_sb = perm.tile([128, NKB, NH, DK + 1], BF16)  # [.., 64] = ones column

    xT = p01.tile([128, 8, S], BF16)     # xT[p, dc, s] = x[s, dc*128+p]
    wqT = p01.tile([128, 8, E], BF16)    # wqT[p, dc, e] = wq[e, dc*128+p]
    wkT = p01.tile([128, 8, E], BF16)
    wvT = p01.tile([128, 8, E], BF16)

    ncopy = [0]

    def copy(dst, src):
        if ncopy[0] % 2 == 0:
            nc.vector.tensor_copy(dst, src)
        else:
            nc.scalar.copy(dst, src)
        ncopy[0] += 1

    with tc.tile_pool(name="ps01", bufs=1, space="PSUM") as ps01:
        warm_ps = ps01.tile([128, 512], F32, tag="warm", bufs=1)
        for _ in range(16):
            nc.tensor.matmul(warm_ps, lhsT=warm_rhs[:, 0:128], rhs=warm_rhs,
                             start=True, stop=True)
        warm_sink = const.tile([128, 4], F32)
        nc.vector.tensor_copy(warm_sink, warm_ps[:, 0:4])

        # ---- phase 0: weight transposes (PE, bf16; interleaved with warm
        # filler matmuls since transpose-mode does not count as PE activity) ----
        nwtp = [0]
        for w_d, wT in [(wq_d, wqT), (wk_d, wkT), (wv_d, wvT)]:
            for ec in range(2):
                w_nat = nat.tile([128, D], BF16, tag="wnat", bufs=2)
                nc.sync.dma_start(out=w_nat, in_=w_d[ec * 128:(ec + 1) * 128, :])
                for dcg in range(2):
                    ps = ps01.tile([128, 512], BF16, tag="tp", bufs=2)
                    for j in range(4):
                        dc = dcg * 4 + j
                        nc.tensor.transpose(
                            ps[:, j * 128:(j + 1) * 128],
                            w_nat[:, dc * 128:(dc + 1) * 128],
                            identity,
                        )
                    copy(wT[:, dcg * 4:(dcg + 1) * 4, ec * 128:(ec + 1) * 128], ps)
                    for _ in range(2):
                        nc.tensor.matmul(warm_ps, lhsT=warm_rhs[:, 0:128],
                                         rhs=warm_rhs, start=True, stop=True)
        # wo [D, E] -> woT [e, dout]
        for dpg in range(2):
            wo_nats = []
            for j in range(4):
                dp = dpg * 4 + j
                wo_nat = nat.tile([128, E], BF16, tag="wonat", bufs=4)
                nc.sync.dma_start(out=wo_nat, in_=wo_d[dp * 128:(dp + 1) * 128, :])
                wo_nats.append(wo_nat)
            for ec in range(2):
                ps = ps01.tile([128, 512], BF16, tag="tp", bufs=2)
                for j in range(4):
                    nc.tensor.transpose(
                        ps[:, j * 128:(j + 1) * 128],
                        wo_nats[j][:, ec * 128:(ec + 1) * 128],
                        identity,
                    )
                copy(woT[:, ec, dpg * 512:(dpg + 1) * 512], ps)
                for _ in range(2):
                    nc.tensor.matmul(warm_ps, lhsT=warm_rhs[:, 0:128],
                                     rhs=warm_rhs, start=True, stop=True)

        # ones column of v (written once; strided 3D AP)
        ones_ap = bass.AP(
            tensor=v_sb.tensor,
            offset=v_sb.offset + DK,
            ap=[v_sb.ap[0], [NH * (DK + 1), NKB], [DK + 1, NH]],
        )
        src64 = bass.AP(
            tensor=ones_f32.tensor, offset=ones_f32.offset,
            ap=[ones_f32.ap[0], [4, NKB], [1, NH]],
        )
        nc.vector.tensor_copy(ones_ap, src64)

        # ---- phase 1: x transposes for chunk sc interleaved with the
        # projection matmuls of chunk sc-1 (keeps HAM warm: transpose-mode
        # does not count as PE activity) ----
        def make_proj_units(sc):
            units = []
            for w_t, outT in [(wqT, qT), (wkT, kT)]:
                for ec in range(2):
                    def u(w_t=w_t, outT=outT, ec=ec, sc=sc):
                        ps = ps01.tile([128, 512], F32, tag="proj", bufs=4,
                                       name="psp")
                        for dc in range(8):
                            nc.tensor.matmul(
                                ps,
                                lhsT=w_t[:, dc, ec * 128:(ec + 1) * 128],
                                rhs=xT[:, dc, sc * 512:(sc + 1) * 512],
                                start=(dc == 0),
                                stop=(dc == 7),
                            )
                        copy(outT[:, ec, sc * 512:(sc + 1) * 512], ps)
                    units.append(u)
            return units

        prev_units = []
        for sc in range(4):
            pu = iter(prev_units)
            for j4 in range(4):
                sblk = sc * 4 + j4
                x_nat = nat.tile([128, D], BF16, tag="xnat", bufs=4)
                nc.sync.dma_start(out=x_nat, in_=x_d[sblk * 128:(sblk + 1) * 128, :])
                for dcg in range(2):
                    ps = ps01.tile([128, 512], BF16, tag="tp", bufs=2)
                    for j in range(4):
                        dc = dcg * 4 + j
                        nc.tensor.transpose(
                            ps[:, j * 128:(j + 1) * 128],
                            x_nat[:, dc * 128:(dc + 1) * 128],
                            identity,
                        )
                    copy(xT[:, dcg * 4:(dcg + 1) * 4, sblk * 128:(sblk + 1) * 128], ps)
                    u = next(pu, None)
                    if u is not None:
                        u()
                    else:
                        # keep the HAM clock-gate warm: transposes do not
                        # count as PE activity
                        for _ in range(3):
                            nc.tensor.matmul(warm_ps, lhsT=warm_rhs[:, 0:128],
                                             rhs=warm_rhs, start=True, stop=True)
            for u in pu:
                u()
            prev_units = make_proj_units(sc)
        for u in prev_units:
            u()

    nat.release()

    work = tc.alloc_tile_pool(name="work", bufs=3)
    small = tc.alloc_tile_pool(name="small", bufs=2)

    # ---- phase 2/3: attention + output projection, work-queue pipelined ----
    with tc.tile_pool(name="psS", bufs=1, space="PSUM") as ps_S, \
         tc.tile_pool(name="psO", bufs=1, space="PSUM") as ps_o, \
         tc.tile_pool(name="psY", bufs=1, space="PSUM") as ps_y:

        workq = []  # (weight, closure) of ready-to-run background PE work

        def make_vproj(sblk):
            def u():
                ps = ps_y.tile([128, E], F32, tag="y", bufs=2, name="psv")
                for dc in range(8):
                    nc.tensor.matmul(
                        ps,
                        lhsT=xT[:, dc, sblk * 128:(sblk + 1) * 128],
                        rhs=wvT[:, dc, :],
                        start=(dc == 0),
                        stop=(dc == 7),
                    )
                # scatter 4 heads into [.., l, 0:64]
                sap = bass.AP(
                    tensor=ps.tensor, offset=ps.offset,
                    ap=[ps.ap[0], [DK, NH], [1, DK]],
                )
                nc.vector.tensor_copy(v_sb[:, sblk, :, 0:DK], sap)
            return u

        for sblk in range(NKB):
            workq.append((2, make_vproj(sblk)))

        def make_av(po_box, pts, kb, hp, kmax, qc):
            def av():
                if po_box[0] is None:
                    po_box[0] = (
                        ps_o.tile([DK + 1, QC], F32, tag="o", bufs=2, name="poA"),
                        ps_o.tile([DK + 1, QC], F32, tag="o", bufs=2, name="poB"),
                    )
                poA, poB = po_box[0]
                pT, cs = pts[kb]
                for hi, po in ((0, poA), (1, poB)):
                    nc.tensor.matmul(
                        po[:, cs:512],
                        lhsT=v_sb[:, kb, 2 * hp + hi, :],
                        rhs=pT[:, hi, cs:512],
                        start=(kb == 0),
                        stop=(kb == kmax - 1),
                    )
            return av

        def make_epi_copies(po_box, sb_box):
            def epi_c():
                poA, poB = po_box[0]
                # free the psum banks quickly with one copy per head
                oA_sb = small.tile([DK + 1, QC], F32R, tag="osb", bufs=4)
                oB_sb = small.tile([DK + 1, QC], F32R, tag="osb", bufs=4)
                nc.vector.tensor_copy(oA_sb, poA)
                nc.vector.tensor_copy(oB_sb, poB)
                sb_box[0] = (oA_sb, oB_sb)
            return epi_c

        def make_epilogue(sb_box, oT, hp):
            def epi():
                oA_sb, oB_sb = sb_box[0]
                for hi, o_sb in ((0, oA_sb), (1, oB_sb)):
                    # broadcast rowsum (row 64) to 64 partitions via K=1
                    # matmul reading partition 64 (row group 64)
                    ps_bc = ps_y.tile([64, QC], F32, tag="y", bufs=2, name="psbc")
                    nc.tensor.matmul(
                        ps_bc,
                        lhsT=ones128[64:65, :],
                        rhs=o_sb[DK:DK + 1, :],
                        start=True,
                        stop=True,
                    )
                    rec = small.tile([64, QC], F32, tag="rec", bufs=2)
                    nc.vector.reciprocal_approx_fast(rec, ps_bc)
                    if hi == 0:
                        nc.vector.tensor_mul(oT[0:64, hp, :], o_sb[0:DK, :], rec)
                    else:
                        tmpB = small.tile([64, QC], BF16, tag="tmpB", bufs=2)
                        nc.vector.tensor_mul(tmpB, o_sb[0:DK, :], rec)
                        # partition shift 0-63 -> 64-127 via sbuf->sbuf DMA
                        nc.sync.dma_start(out=oT[64:128, hp, :], in_=tmpB)
            return epi

        def make_out_proj(qc, oT):
            units = []
            for dc in range(8):
                def u(dc=dc, qc=qc, oT=oT):
                    psy = ps_y.tile([128, QC], F32, tag="y", bufs=2, name="psy")
                    for ec in range(2):
                        nc.tensor.matmul(
                            psy,
                            lhsT=woT[:, ec, dc * 128:(dc + 1) * 128],
                            rhs=oT[:, ec, :],
                            start=(ec == 0),
                            stop=(ec == 1),
                        )
                    y_sb = work.tile([128, QC], F32, tag="ysb", bufs=3)
                    nc.vector.tensor_copy(y_sb, psy)
                    nc.sync.dma_start(
                        out=yT_d[dc * 128:(dc + 1) * 128, qc * QC:(qc + 1) * QC],
                        in_=y_sb,
                    )
                units.append(u)
            return units

        pending_op = []
        for qc in range(NQC - 1, -1, -1):
            oT = work.tile([128, 2, QC], BF16, tag="oT", bufs=2)
            kmax = 4 * (qc + 1)
            for hp in range(2):
                pts = {}
                po_box = [None]
                sb_box = [None]
                for kb in range(kmax):
                    # S^T = k q^T, 2-head row-tiled pair, causally narrowed
                    cs = max(0, kb * 128 - qc * QC)
                    psS = ps_S.tile([128, 2, 512], F32, tag="S", bufs=2)
                    for hi in range(2):
                        nc.tensor.matmul(
                            psS[:, hi, cs:512],
                            lhsT=kT[hi * 64:(hi + 1) * 64, hp,
                                    kb * 128:(kb + 1) * 128],
                            rhs=qT[hi * 64:(hi + 1) * 64, hp,
                                   qc * QC + cs:(qc + 1) * QC],
                            start=True,
                            stop=True,
                        )
                    if kb >= 4 * qc:  # diagonal band: triangular mask
                        for hi in range(2):
                            nc.vector.tensor_add(
                                psS[:, hi, cs:cs + 128],
                                psS[:, hi, cs:cs + 128],
                                mask,
                            )
                    pT = work.tile([128, 2, 512], BF16, tag="pT", bufs=24)
                    pts[kb] = (pT, cs)
                    nc.scalar.activation(
                        pT[:, :, cs:512],
                        psS[:, :, cs:512],
                        mybir.ActivationFunctionType.Exp,
                        scale=SCALE,
                    )
                    budget = 2 if len(workq) > 12 else 1
                    while workq and budget > 0:
                        w, u = workq.pop(0)
                        u()
                        budget -= w
                # queue this phase's AV work + epilogue
                for kb in range(kmax):
                    workq.append((1, make_av(po_box, pts, kb, hp, kmax, qc)))
                workq.append((0, make_epi_copies(po_box, sb_box)))
                workq.append((1, make_epilogue(sb_box, oT, hp)))
                workq.extend(pending_op)
                pending_op = []
                if hp == 1:
                    pending_op = [(1, u) for u in make_out_proj(qc, oT)]
        for _, u in workq + pending_op:
            u()

    for p in [small, work, p01, perm, const]:
        p.release()


_CACHE = {}


def _build():
    if "nc" in _CACHE:
        return _CACHE["nc"]
    nc = bacc.Bacc("TRN2", target_bir_lowering=False, debug=False, num_devices=8)
    x_d = nc.dram_tensor("x", [S, D], BF16, kind="ExternalInput").ap()
    wq_d = nc.dram_tensor("wq", [E, D], BF16, kind="ExternalInput").ap()
    wk_d = nc.dram_tensor("wk", [E, D], BF16, kind="ExternalInput").ap()
    wv_d = nc.dram_tensor("wv", [E, D], BF16, kind="ExternalInput").ap()
    wo_d = nc.dram_tensor("wo", [D, E], BF16, kind="ExternalInput").ap()
    yT_d = nc.dram_tensor("yT", [D, S], F32, kind="ExternalOutput").ap()
    with tile.TileContext(nc) as tc:
        _emit(tc, nc, x_d, wq_d, wk_d, wv_d, wo_d, yT_d)
    nc.compile()
    _CACHE["nc"] = nc
    return nc


LAST_RESULT = None


def kernel(x, wq, wk, wv, wo):
    global LAST_RESULT
    nc = _build()
    bf = ml_dtypes.bfloat16
    x = np.asarray(x, dtype=np.float32)
    wq16 = np.asarray(wq, dtype=np.float32).astype(bf)
    wk16 = np.asarray(wk, dtype=np.float32).astype(bf)
    wv16 = np.asarray(wv, dtype=np.float32).astype(bf)
    wo16 = np.asarray(wo, dtype=np.float32).astype(bf)

    in_maps = []
    for c in range(8):
        b, g = c // 4, c % 4
        rows = slice(g * E, (g + 1) * E)
        in_maps.append({
            "x": np.ascontiguousarray(x[b].astype(bf)),
            "wq": np.ascontiguousarray(wq16[rows]),
            "wk": np.ascontiguousarray(wk16[rows]),
            "wv": np.ascontiguousarray(wv16[rows]),
            "wo": np.ascontiguousarray(wo16[:, rows]),
        })

    res = bass_utils.run_bass_kernel_spmd(nc, in_maps, core_ids=list(range(8)))
    LAST_RESULT = res

    y = np.empty((B, S, D), dtype=np.float32)
    for b in range(B):
        acc = res.results[4 * b]["yT"].copy()
        for g in range(1, 4):
            acc += res.results[4 * b + g]["yT"]
        y[b] = acc.T
    return y


# revision 20
# speedup vs baseline: 1.3100x; 1.0116x over previous
"""Causal multi-head attention on 8 Trainium2 NeuronCores.

Sharding: data-parallel over batch (B=2) x tensor-parallel over heads
(16 heads -> 4 groups of 4). Core c handles batch c//4, head group c%4.
Each core computes q/k/v projections for its 4 heads, causal flash
attention, and a partial output projection (row slice of Wo); the host
sums the 4 partials per batch element.

Matmuls run in bf16 (fp32 PSUM accumulation): the PE streams 2-byte
moving operands at 1 cycle/row vs 2 for 4-byte. Inputs are cast to bf16
on the host. The softmax row-sum is fused into the o^T = [v|1s]^T P^T
matmul via an appended ones column; normalization (broadcast rowsum via
K=1 matmul reading partition 64, fast-approx reciprocal, divide) stays
in fp32.

To keep the PE dense (and its HAM clock-gate at 8/8), both big phases
are software-pipelined at the instruction-emission level:
  phase 1: x-transpose groups for s-chunk sc interleave with the q/k/v
           projection matmuls of s-chunk sc-1.
  phase 2: per (q-chunk, head-pair) only the S^T = k q^T matmuls and the
           masked exp are emitted in the main loop; AV matmuls,
           normalization epilogue, and the previous q-chunk's output
           projection drain from a work queue between them.
"""

import numpy as np
import ml_dtypes

import concourse.bacc as bacc
import concourse.bass as bass
import concourse.tile as tile
from concourse import bass_utils, mybir
from concourse.masks import make_identity

B, S, D, H = 2, 2048, 1024, 16
DK = 64
NH = 4                 # heads per core
E = NH * DK            # 256: per-core head-dim slice
SCALE = 1.0 / 8.0      # 1/sqrt(DK)
NEG = -30000.0

F32 = mybir.dt.float32
F32R = mybir.dt.float32r
BF16 = mybir.dt.bfloat16

QC = 512               # q-chunk (columns per attention tile)
NQC = S // QC          # 4
NKB = S // 128         # 16 k-blocks


def _emit(tc, nc, x_d, wq_d, wk_d, wv_d, wo_d, yT_d):
    const = tc.alloc_tile_pool(name="const", bufs=1)
    perm = tc.alloc_tile_pool(name="perm", bufs=1)
    p01 = tc.alloc_tile_pool(name="p01", bufs=1)
    nat = tc.alloc_tile_pool(name="nat", bufs=4)

    # HAM warmup: back-to-back matmuls on zeros flip the PE clock gate to
    # 8/8 within ~3.4us so phase 0/1 runs at 2.4 GHz.
    warm_rhs = const.tile([128, 512], BF16)
    nc.vector.memset(warm_rhs, 0.0)
    identity = const.tile([128, 128], BF16)
    make_identity(nc, identity)
    # ones row for the rowsum broadcast (row 64 used as lhsT)
    ones128 = const.tile([128, 64], F32R)
    ones_f32 = const.tile([128, 64], F32)
    nc.gpsimd.memset(ones_f32, 1.0)
    nc.vector.tensor_copy(ones128, ones_f32)
    # causal triangle mask for diagonal 128x128 blocks of S^T
    # (partition r = key index, free c = query index): keep 0 where r <= c,
    # else a large negative so exp() underflows to exactly 0.
    mask = const.tile([128, 128], F32)
    nc.gpsimd.memset(mask, 0.0)
    # out[r, c] = (c - r) >= 0 ? 0.0 : NEG
    nc.gpsimd.affine_select(
        out=mask,
        in_=mask,
        compare_op=mybir.AluOpType.is_ge,
        fill=NEG,
        base=0,
        pattern=[[1, 128]],
        channel_multiplier=-1,
    )

    woT = perm.tile([128, 2, D], BF16)   # woT[p, ec, o] = wo[o, ec*128+p]
    qT = perm.tile([128, 2, S], BF16)    # qT[p, ec, s] = q[s, ec*128+p]
    kT = perm.tile([128, 2, S], BF16)
    v_sb = perm.tile([128, NKB, NH, DK + 1], BF16)  # [.., 64] = ones column

    xT = p01.tile([128, 8, S], BF16)     # xT[p, dc, s] = x[s, dc*128+p]
    wqT = p01.tile([128, 8, E], BF16)    # wqT[p, dc, e] = wq[e, dc*128+p]
    wkT = p01.tile([128, 8, E], BF16)
    wvT = p01.tile([128, 8, E], BF16)

    ncopy = [0]

    def copy(dst, src):
        if ncopy[0] % 2 == 0:
            nc.vector.tensor_copy(dst, src)
        else:
            nc.scalar.copy(dst, src)
        ncopy[0] += 1

    with tc.tile_pool(name="ps01", bufs=1, space="PSUM") as ps01:
        warm_ps = ps01.tile([128, 512], F32, tag="warm", bufs=1)
        for _ in range(16):
            nc.tensor.matmul(warm_ps, lhsT=warm_rhs[:, 0:128], rhs=warm_rhs,
                             start=True, stop=True)
        warm_sink = const.tile([128, 4], F32)
        nc.vector.tensor_copy(warm_sink, warm_ps[:, 0:4])

        # ---- phase 0: weight transposes (PE, bf16: 1 cycle/row) ----
        for w_d, wT in [(wq_d, wqT), (wk_d, wkT), (wv_d, wvT)]:
            for ec in range(2):
                w_nat = nat.tile([128, D], BF16, tag="wnat", bufs=2)
                nc.sync.dma_start(out=w_nat, in_=w_d[ec * 128:(ec + 1) * 128, :])
                for dcg in range(2):
                    ps = ps01.tile([128, 512], BF16, tag="tp", bufs=2)
                    for j in range(4):
                        dc = dcg * 4 + j
                        nc.tensor.transpose(
                            ps[:, j * 128:(j + 1) * 128],
                            w_nat[:, dc * 128:(dc + 1) * 128],
                            identity,
                        )
                    copy(wT[:, dcg * 4:(dcg + 1) * 4, ec * 128:(ec + 1) * 128], ps)
        # wo [D, E] -> woT [e, dout]
        for dpg in range(2):
            wo_nats = []
            for j in range(4):
                dp = dpg * 4 + j
                wo_nat = nat.tile([128, E], BF16, tag="wonat", bufs=4)
                nc.sync.dma_start(out=wo_nat, in_=wo_d[dp * 128:(dp + 1) * 128, :])
                wo_nats.append(wo_nat)
            for ec in range(2):
                ps = ps01.tile([128, 512], BF16, tag="tp", bufs=2)
                for j in range(4):
                    nc.tensor.transpose(
                        ps[:, j * 128:(j + 1) * 128],
                        wo_nats[j][:, ec * 128:(ec + 1) * 128],
                        identity,
                    )
                copy(woT[:, ec, dpg * 512:(dpg + 1) * 512], ps)

        # ones column of v (written once; strided 3D AP)
        ones_ap = bass.AP(
            tensor=v_sb.tensor,
            offset=v_sb.offset + DK,
            ap=[v_sb.ap[0], [NH * (DK + 1), NKB], [DK + 1, NH]],
        )
        src64 = bass.AP(
            tensor=ones_f32.tensor, offset=ones_f32.offset,
            ap=[ones_f32.ap[0], [4, NKB], [1, NH]],
        )
        nc.vector.tensor_copy(ones_ap, src64)

        # ---- phase 1: x transposes for chunk sc interleaved with the
        # projection matmuls of chunk sc-1 (keeps HAM warm: transpose-mode
        # does not count as PE activity) ----
        def make_proj_units(sc):
            units = []
            for w_t, outT in [(wqT, qT), (wkT, kT)]:
                for ec in range(2):
                    def u(w_t=w_t, outT=outT, ec=ec, sc=sc):
                        ps = ps01.tile([128, 512], F32, tag="proj", bufs=4,
                                       name="psp")
                        for dc in range(8):
                            nc.tensor.matmul(
                                ps,
                                lhsT=w_t[:, dc, ec * 128:(ec + 1) * 128],
                                rhs=xT[:, dc, sc * 512:(sc + 1) * 512],
                                start=(dc == 0),
                                stop=(dc == 7),
                            )
                        copy(outT[:, ec, sc * 512:(sc + 1) * 512], ps)
                    units.append(u)
            return units

        prev_units = []
        for sc in range(4):
            pu = iter(prev_units)
            for j4 in range(4):
                sblk = sc * 4 + j4
                x_nat = nat.tile([128, D], BF16, tag="xnat", bufs=4)
                nc.sync.dma_start(out=x_nat, in_=x_d[sblk * 128:(sblk + 1) * 128, :])
                for dcg in range(2):
                    ps = ps01.tile([128, 512], BF16, tag="tp", bufs=2)
                    for j in range(4):
                        dc = dcg * 4 + j
                        nc.tensor.transpose(
                            ps[:, j * 128:(j + 1) * 128],
                            x_nat[:, dc * 128:(dc + 1) * 128],
                            identity,
                        )
                    copy(xT[:, dcg * 4:(dcg + 1) * 4, sblk * 128:(sblk + 1) * 128], ps)
                    u = next(pu, None)
                    if u is not None:
                        u()
            for u in pu:
                u()
            prev_units = make_proj_units(sc)
        for u in prev_units:
            u()

    nat.release()

    work = tc.alloc_tile_pool(name="work", bufs=3)
    small = tc.alloc_tile_pool(name="small", bufs=2)

    # ---- phase 2/3: attention + output projection, work-queue pipelined ----
    with tc.tile_pool(name="psS", bufs=1, space="PSUM") as ps_S, \
         tc.tile_pool(name="psO", bufs=1, space="PSUM") as ps_o, \
         tc.tile_pool(name="psY", bufs=1, space="PSUM") as ps_y:

        workq = []  # (weight, closure) of ready-to-run background PE work

        def make_vproj(sblk):
            def u():
                ps = ps_y.tile([128, E], F32, tag="y", bufs=2, name="psv")
                for dc in range(8):
                    nc.tensor.matmul(
                        ps,
                        lhsT=xT[:, dc, sblk * 128:(sblk + 1) * 128],
                        rhs=wvT[:, dc, :],
                        start=(dc == 0),
                        stop=(dc == 7),
                    )
                # scatter 4 heads into [.., l, 0:64]
                sap = bass.AP(
                    tensor=ps.tensor, offset=ps.offset,
                    ap=[ps.ap[0], [DK, NH], [1, DK]],
                )
                nc.vector.tensor_copy(v_sb[:, sblk, :, 0:DK], sap)
            return u

        for sblk in range(NKB):
            workq.append((2, make_vproj(sblk)))

        def make_av(po_box, pts, kb, hp, kmax, qc):
            def av():
                if po_box[0] is None:
                    po_box[0] = (
                        ps_o.tile([DK + 1, QC], F32, tag="o", bufs=2, name="poA"),
                        ps_o.tile([DK + 1, QC], F32, tag="o", bufs=2, name="poB"),
                    )
                poA, poB = po_box[0]
                pT, cs = pts[kb]
                for hi, po in ((0, poA), (1, poB)):
                    nc.tensor.matmul(
                        po[:, cs:512],
                        lhsT=v_sb[:, kb, 2 * hp + hi, :],
                        rhs=pT[:, hi, cs:512],
                        start=(kb == 0),
                        stop=(kb == kmax - 1),
                    )
            return av

        def make_epilogue(po_box, oT, hp):
            def epi():
                poA, poB = po_box[0]
                # free the psum banks quickly with one copy per head
                oA_sb = small.tile([DK + 1, QC], F32R, tag="osb", bufs=4)
                oB_sb = small.tile([DK + 1, QC], F32R, tag="osb", bufs=4)
                nc.vector.tensor_copy(oA_sb, poA)
                nc.vector.tensor_copy(oB_sb, poB)
                for hi, o_sb in ((0, oA_sb), (1, oB_sb)):
                    # broadcast rowsum (row 64) to 64 partitions via K=1
                    # matmul reading partition 64 (row group 64)
                    ps_bc = ps_y.tile([64, QC], F32, tag="y", bufs=2, name="psbc")
                    nc.tensor.matmul(
                        ps_bc,
                        lhsT=ones128[64:65, :],
                        rhs=o_sb[DK:DK + 1, :],
                        start=True,
                        stop=True,
                    )
                    rec = small.tile([64, QC], F32, tag="rec", bufs=2)
                    nc.vector.reciprocal_approx_fast(rec, ps_bc)
                    if hi == 0:
                        nc.vector.tensor_mul(oT[0:64, hp, :], o_sb[0:DK, :], rec)
                    else:
                        tmpB = small.tile([64, QC], BF16, tag="tmpB", bufs=2)
                        nc.vector.tensor_mul(tmpB, o_sb[0:DK, :], rec)
                        # partition shift 0-63 -> 64-127 via sbuf->sbuf DMA
                        nc.sync.dma_start(out=oT[64:128, hp, :], in_=tmpB)
            return epi

        def make_out_proj(qc, oT):
            units = []
            for dc in range(8):
                def u(dc=dc, qc=qc, oT=oT):
                    psy = ps_y.tile([128, QC], F32, tag="y", bufs=2, name="psy")
                    for ec in range(2):
                        nc.tensor.matmul(
                            psy,
                            lhsT=woT[:, ec, dc * 128:(dc + 1) * 128],
                            rhs=oT[:, ec, :],
                            start=(ec == 0),
                            stop=(ec == 1),
                        )
                    y_sb = work.tile([128, QC], F32, tag="ysb", bufs=3)
                    nc.vector.tensor_copy(y_sb, psy)
                    nc.sync.dma_start(
                        out=yT_d[dc * 128:(dc + 1) * 128, qc * QC:(qc + 1) * QC],
                        in_=y_sb,
                    )
                units.append(u)
            return units

        for qc in range(NQC - 1, -1, -1):
            oT = work.tile([128, 2, QC], BF16, tag="oT", bufs=2)
            kmax = 4 * (qc + 1)
            for hp in range(2):
                pts = {}
                po_box = [None]
                for kb in range(kmax):
                    # S^T = k q^T, 2-head row-tiled pair, causally narrowed
                    cs = max(0, kb * 128 - qc * QC)
                    psS = ps_S.tile([128, 2, 512], F32, tag="S", bufs=2)
                    for hi in range(2):
                        nc.tensor.matmul(
                            psS[:, hi, cs:512],
                            lhsT=kT[hi * 64:(hi + 1) * 64, hp,
                                    kb * 128:(kb + 1) * 128],
                            rhs=qT[hi * 64:(hi + 1) * 64, hp,
                                   qc * QC + cs:(qc + 1) * QC],
                            start=True,
                            stop=True,
                        )
                    if kb >= 4 * qc:  # diagonal band: triangular mask
                        for hi in range(2):
                            nc.vector.tensor_add(
                                psS[:, hi, cs:cs + 128],
                                psS[:, hi, cs:cs + 128],
                                mask,
                            )
                    pT = work.tile([128, 2, 512], BF16, tag="pT", bufs=24)
                    pts[kb] = (pT, cs)
                    nc.scalar.activation(
                        pT[:, :, cs:512],
                        psS[:, :, cs:512],
                        mybir.ActivationFunctionType.Exp,
                        scale=SCALE,
                    )
                    budget = 2 if len(workq) > 12 else 1
                    while workq and budget > 0:
                        w, u = workq.pop(0)
                        u()
                        budget -= w
                # queue this phase's AV work + epilogue
                for kb in range(kmax):
                    workq.append((1, make_av(po_box, pts, kb, hp, kmax, qc)))
                workq.append((1, make_epilogue(po_box, oT, hp)))
                if hp == 1:
                    workq.extend((1, u) for u in make_out_proj(qc, oT))
        for _, u in workq:
            u()

    for p in [small, work, p01, perm, const]:
        p.release()


_CACHE = {}


def _build():
    if "nc" in _CACHE:
        return _CACHE["nc"]
    nc = bacc.Bacc("TRN2", target_bir_lowering=False, debug=False, num_devices=8)
    x_d = nc.dram_tensor("x", [S, D], BF16, kind="ExternalInput").ap()
    wq_d = nc.dram_tensor("wq", [E, D], BF16, kind="ExternalInput").ap()
    wk_d = nc.dram_tensor("wk", [E, D], BF16, kind="ExternalInput").ap()
    wv_d = nc.dram_tensor("wv", [E, D], BF16, kind="ExternalInput").ap()
    wo_d = nc.dram_tensor("wo", [D, E], BF16, kind="ExternalInput").ap()
    yT_d = nc.dram_tensor("yT", [D, S], F32, kind="ExternalOutput").ap()
    with tile.TileContext(nc) as tc:
        _emit(tc, nc, x_d, wq_d, wk_d, wv_d, wo_d, yT_d)
    nc.compile()
    _CACHE["nc"] = nc
    return nc


LAST_RESULT = None


def kernel(x, wq, wk, wv, wo):
    global LAST_RESULT
    nc = _build()
    bf = ml_dtypes.bfloat16
    x = np.asarray(x, dtype=np.float32)
    wq16 = np.asarray(wq, dtype=np.float32).astype(bf)
    wk16 = np.asarray(wk, dtype=np.float32).astype(bf)
    wv16 = np.asarray(wv, dtype=np.float32).astype(bf)
    wo16 = np.asarray(wo, dtype=np.float32).astype(bf)

    in_maps = []
    for c in range(8):
        b, g = c // 4, c % 4
        rows = slice(g * E, (g + 1) * E)
        in_maps.append({
            "x": np.ascontiguousarray(x[b].astype(bf)),
            "wq": np.ascontiguousarray(wq16[rows]),
            "wk": np.ascontiguousarray(wk16[rows]),
            "wv": np.ascontiguousarray(wv16[rows]),
            "wo": np.ascontiguousarray(wo16[:, rows]),
        })

    res = bass_utils.run_bass_kernel_spmd(nc, in_maps, core_ids=list(range(8)))
    LAST_RESULT = res

    y = np.empty((B, S, D), dtype=np.float32)
    for b in range(B):
        acc = res.results[4 * b]["yT"].copy()
        for g in range(1, 4):
            acc += res.results[4 * b + g]["yT"]
        y[b] = acc.T
    return y


# revision 21
# speedup vs baseline: 1.3761x; 1.0504x over previous
"""Causal multi-head attention on 8 Trainium2 NeuronCores.

Sharding: data-parallel over batch (B=2) x tensor-parallel over heads
(16 heads -> 4 groups of 4). Core c handles batch c//4, head group c%4.
Each core computes q/k/v projections for its 4 heads, causal flash
attention, and a partial output projection (row slice of Wo); the host
sums the 4 partials per batch element.

Matmuls run in bf16 (fp32 PSUM accumulation): the PE streams 2-byte
moving operands at 1 cycle/row vs 2 for 4-byte. Inputs are cast to bf16
on the host. The softmax row-sum is fused into the o^T = [v|1s]^T P^T
matmul via an appended ones column; normalization (broadcast rowsum via
K=1 matmul reading partition 64, fast-approx reciprocal, divide) stays
in fp32.

To keep the PE dense (and its HAM clock-gate at 8/8), both big phases
are software-pipelined at the instruction-emission level:
  phase 1: x-transpose groups for s-chunk sc interleave with the q/k/v
           projection matmuls of s-chunk sc-1.
  phase 2: per (q-chunk, head-pair) only the S^T = k q^T matmuls and the
           masked exp are emitted in the main loop; AV matmuls,
           normalization epilogue, and the previous q-chunk's output
           projection drain from a work queue between them.
"""

import numpy as np
import ml_dtypes

import concourse.bacc as bacc
import concourse.bass as bass
import concourse.tile as tile
from concourse import bass_utils, mybir
from concourse.masks import make_identity

B, S, D, H = 2, 2048, 1024, 16
DK = 64
NH = 4                 # heads per core
E = NH * DK            # 256: per-core head-dim slice
SCALE = 1.0 / 8.0      # 1/sqrt(DK)
NEG = -30000.0

F32 = mybir.dt.float32
F32R = mybir.dt.float32r
BF16 = mybir.dt.bfloat16

QC = 512               # q-chunk (columns per attention tile)
NQC = S // QC          # 4
NKB = S // 128         # 16 k-blocks


def _emit(tc, nc, x_d, wq_d, wk_d, wv_d, wo_d, yT_d):
    const = tc.alloc_tile_pool(name="const", bufs=1)
    perm = tc.alloc_tile_pool(name="perm", bufs=1)
    p01 = tc.alloc_tile_pool(name="p01", bufs=1)
    nat = tc.alloc_tile_pool(name="nat", bufs=4)

    identity = const.tile([128, 128], BF16)
    make_identity(nc, identity)
    # ones row for the rowsum broadcast (row 64 used as lhsT)
    ones128 = const.tile([128, 64], F32R)
    ones_f32 = const.tile([128, 64], F32)
    nc.gpsimd.memset(ones_f32, 1.0)
    nc.vector.tensor_copy(ones128, ones_f32)
    # causal triangle mask for diagonal 128x128 blocks of S^T
    # (partition r = key index, free c = query index): keep 0 where r <= c,
    # else a large negative so exp() underflows to exactly 0.
    mask = const.tile([128, 128], F32)
    nc.gpsimd.memset(mask, 0.0)
    # out[r, c] = (c - r) >= 0 ? 0.0 : NEG
    nc.gpsimd.affine_select(
        out=mask,
        in_=mask,
        compare_op=mybir.AluOpType.is_ge,
        fill=NEG,
        base=0,
        pattern=[[1, 128]],
        channel_multiplier=-1,
    )

    woT = perm.tile([128, 2, D], BF16)   # woT[p, ec, o] = wo[o, ec*128+p]
    qT = perm.tile([128, 2, S], BF16)    # qT[p, ec, s] = q[s, ec*128+p]
    kT = perm.tile([128, 2, S], BF16)
    v_sb = perm.tile([128, NKB, NH, DK + 1], BF16)  # [.., 64] = ones column

    xT = p01.tile([128, 8, S], BF16)     # xT[p, dc, s] = x[s, dc*128+p]
    wqT = p01.tile([128, 8, E], BF16)    # wqT[p, dc, e] = wq[e, dc*128+p]
    wkT = p01.tile([128, 8, E], BF16)
    wvT = p01.tile([128, 8, E], BF16)

    ncopy = [0]

    def copy(dst, src):
        if ncopy[0] % 2 == 0:
            nc.vector.tensor_copy(dst, src)
        else:
            nc.scalar.copy(dst, src)
        ncopy[0] += 1

    with tc.tile_pool(name="ps01", bufs=1, space="PSUM") as ps01:
        # ---- phase 0: weight transposes (PE, bf16: 1 cycle/row) ----
        for w_d, wT in [(wq_d, wqT), (wk_d, wkT), (wv_d, wvT)]:
            for ec in range(2):
                w_nat = nat.tile([128, D], BF16, tag="wnat", bufs=2)
                nc.sync.dma_start(out=w_nat, in_=w_d[ec * 128:(ec + 1) * 128, :])
                for dcg in range(2):
                    ps = ps01.tile([128, 512], BF16, tag="tp", bufs=2)
                    for j in range(4):
                        dc = dcg * 4 + j
                        nc.tensor.transpose(
                            ps[:, j * 128:(j + 1) * 128],
                            w_nat[:, dc * 128:(dc + 1) * 128],
                            identity,
                        )
                    copy(wT[:, dcg * 4:(dcg + 1) * 4, ec * 128:(ec + 1) * 128], ps)
        # wo [D, E] -> woT [e, dout]
        for dpg in range(2):
            wo_nats = []
            for j in range(4):
                dp = dpg * 4 + j
                wo_nat = nat.tile([128, E], BF16, tag="wonat", bufs=4)
                nc.sync.dma_start(out=wo_nat, in_=wo_d[dp * 128:(dp + 1) * 128, :])
                wo_nats.append(wo_nat)
            for ec in range(2):
                ps = ps01.tile([128, 512], BF16, tag="tp", bufs=2)
                for j in range(4):
                    nc.tensor.transpose(
                        ps[:, j * 128:(j + 1) * 128],
                        wo_nats[j][:, ec * 128:(ec + 1) * 128],
                        identity,
                    )
                copy(woT[:, ec, dpg * 512:(dpg + 1) * 512], ps)

        # ones column of v (written once; strided 3D AP)
        ones_ap = bass.AP(
            tensor=v_sb.tensor,
            offset=v_sb.offset + DK,
            ap=[v_sb.ap[0], [NH * (DK + 1), NKB], [DK + 1, NH]],
        )
        src64 = bass.AP(
            tensor=ones_f32.tensor, offset=ones_f32.offset,
            ap=[ones_f32.ap[0], [4, NKB], [1, NH]],
        )
        nc.vector.tensor_copy(ones_ap, src64)

        # ---- phase 1: x transposes for chunk sc interleaved with the
        # projection matmuls of chunk sc-1 (keeps HAM warm: transpose-mode
        # does not count as PE activity) ----
        def make_proj_units(sc):
            units = []
            for w_t, outT in [(wqT, qT), (wkT, kT)]:
                for ec in range(2):
                    def u(w_t=w_t, outT=outT, ec=ec, sc=sc):
                        ps = ps01.tile([128, 512], F32, tag="proj", bufs=4,
                                       name="psp")
                        for dc in range(8):
                            nc.tensor.matmul(
                                ps,
                                lhsT=w_t[:, dc, ec * 128:(ec + 1) * 128],
                                rhs=xT[:, dc, sc * 512:(sc + 1) * 512],
                                start=(dc == 0),
                                stop=(dc == 7),
                            )
                        copy(outT[:, ec, sc * 512:(sc + 1) * 512], ps)
                    units.append(u)
            return units

        prev_units = []
        for sc in range(4):
            pu = iter(prev_units)
            for j4 in range(4):
                sblk = sc * 4 + j4
                x_nat = nat.tile([128, D], BF16, tag="xnat", bufs=4)
                nc.sync.dma_start(out=x_nat, in_=x_d[sblk * 128:(sblk + 1) * 128, :])
                for dcg in range(2):
                    ps = ps01.tile([128, 512], BF16, tag="tp", bufs=2)
                    for j in range(4):
                        dc = dcg * 4 + j
                        nc.tensor.transpose(
                            ps[:, j * 128:(j + 1) * 128],
                            x_nat[:, dc * 128:(dc + 1) * 128],
                            identity,
                        )
                    copy(xT[:, dcg * 4:(dcg + 1) * 4, sblk * 128:(sblk + 1) * 128], ps)
                    u = next(pu, None)
                    if u is not None:
                        u()
            for u in pu:
                u()
            prev_units = make_proj_units(sc)
        for u in prev_units:
            u()

    nat.release()

    work = tc.alloc_tile_pool(name="work", bufs=3)
    small = tc.alloc_tile_pool(name="small", bufs=2)

    # ---- phase 2/3: attention + output projection, work-queue pipelined ----
    with tc.tile_pool(name="psS", bufs=1, space="PSUM") as ps_S, \
         tc.tile_pool(name="psO", bufs=1, space="PSUM") as ps_o, \
         tc.tile_pool(name="psY", bufs=1, space="PSUM") as ps_y:

        workq = []  # (weight, closure) of ready-to-run background PE work

        def make_vproj(sblk):
            def u():
                ps = ps_y.tile([128, E], F32, tag="y", bufs=2, name="psv")
                for dc in range(8):
                    nc.tensor.matmul(
                        ps,
                        lhsT=xT[:, dc, sblk * 128:(sblk + 1) * 128],
                        rhs=wvT[:, dc, :],
                        start=(dc == 0),
                        stop=(dc == 7),
                    )
                # scatter 4 heads into [.., l, 0:64]
                sap = bass.AP(
                    tensor=ps.tensor, offset=ps.offset,
                    ap=[ps.ap[0], [DK, NH], [1, DK]],
                )
                nc.vector.tensor_copy(v_sb[:, sblk, :, 0:DK], sap)
            return u

        for sblk in range(NKB):
            workq.append((2, make_vproj(sblk)))

        def make_av(po_box, pts, kb, hp, kmax, qc):
            def av():
                if po_box[0] is None:
                    po_box[0] = (
                        ps_o.tile([DK + 1, QC], F32, tag="o", bufs=2, name="poA"),
                        ps_o.tile([DK + 1, QC], F32, tag="o", bufs=2, name="poB"),
                    )
                poA, poB = po_box[0]
                pT, cs = pts[kb]
                for hi, po in ((0, poA), (1, poB)):
                    nc.tensor.matmul(
                        po[:, cs:512],
                        lhsT=v_sb[:, kb, 2 * hp + hi, :],
                        rhs=pT[:, hi, cs:512],
                        start=(kb == 0),
                        stop=(kb == kmax - 1),
                    )
            return av

        def make_epilogue(po_box, oT, hp):
            def epi():
                poA, poB = po_box[0]
                # free the psum banks quickly with one copy per head
                oA_sb = small.tile([DK + 1, QC], F32R, tag="osb", bufs=4)
                oB_sb = small.tile([DK + 1, QC], F32R, tag="osb", bufs=4)
                nc.vector.tensor_copy(oA_sb, poA)
                nc.vector.tensor_copy(oB_sb, poB)
                for hi, o_sb in ((0, oA_sb), (1, oB_sb)):
                    # broadcast rowsum (row 64) to 64 partitions via K=1
                    # matmul reading partition 64 (row group 64)
                    ps_bc = ps_y.tile([64, QC], F32, tag="y", bufs=2, name="psbc")
                    nc.tensor.matmul(
                        ps_bc,
                        lhsT=ones128[64:65, :],
                        rhs=o_sb[DK:DK + 1, :],
                        start=True,
                        stop=True,
                    )
                    rec = small.tile([64, QC], F32, tag="rec", bufs=2)
                    nc.vector.reciprocal_approx_fast(rec, ps_bc)
                    if hi == 0:
                        nc.vector.tensor_mul(oT[0:64, hp, :], o_sb[0:DK, :], rec)
                    else:
                        tmpB = small.tile([64, QC], BF16, tag="tmpB", bufs=2)
                        nc.vector.tensor_mul(tmpB, o_sb[0:DK, :], rec)
                        # partition shift 0-63 -> 64-127 via sbuf->sbuf DMA
                        nc.sync.dma_start(out=oT[64:128, hp, :], in_=tmpB)
            return epi

        def make_out_proj(qc, oT):
            units = []
            for dc in range(8):
                def u(dc=dc, qc=qc, oT=oT):
                    psy = ps_y.tile([128, QC], F32, tag="y", bufs=2, name="psy")
                    for ec in range(2):
                        nc.tensor.matmul(
                            psy,
                            lhsT=woT[:, ec, dc * 128:(dc + 1) * 128],
                            rhs=oT[:, ec, :],
                            start=(ec == 0),
                            stop=(ec == 1),
                        )
                    y_sb = work.tile([128, QC], F32, tag="ysb", bufs=3)
                    nc.vector.tensor_copy(y_sb, psy)
                    nc.sync.dma_start(
                        out=yT_d[dc * 128:(dc + 1) * 128, qc * QC:(qc + 1) * QC],
                        in_=y_sb,
                    )
                units.append(u)
            return units

        for qc in range(NQC - 1, -1, -1):
            oT = work.tile([128, 2, QC], BF16, tag="oT", bufs=2)
            kmax = 4 * (qc + 1)
            for hp in range(2):
                pts = {}
                po_box = [None]
                for kb in range(kmax):
                    # S^T = k q^T, 2-head row-tiled pair, causally narrowed
                    cs = max(0, kb * 128 - qc * QC)
                    psS = ps_S.tile([128, 2, 512], F32, tag="S", bufs=2)
                    for hi in range(2):
                        nc.tensor.matmul(
                            psS[:, hi, cs:512],
                            lhsT=kT[hi * 64:(hi + 1) * 64, hp,
                                    kb * 128:(kb + 1) * 128],
                            rhs=qT[hi * 64:(hi + 1) * 64, hp,
                                   qc * QC + cs:(qc + 1) * QC],
                            start=True,
                            stop=True,
                        )
                    if kb >= 4 * qc:  # diagonal band: triangular mask
                        for hi in range(2):
                            nc.vector.tensor_add(
                                psS[:, hi, cs:cs + 128],
                                psS[:, hi, cs:cs + 128],
                                mask,
                            )
                    pT = work.tile([128, 2, 512], BF16, tag="pT", bufs=24)
                    pts[kb] = (pT, cs)
                    nc.scalar.activation(
                        pT[:, :, cs:512],
                        psS[:, :, cs:512],
                        mybir.ActivationFunctionType.Exp,
                        scale=SCALE,
                    )
                    budget = 2 if len(workq) > 12 else 1
                    while workq and budget > 0:
                        w, u = workq.pop(0)
                        u()
                        budget -= w
                # queue this phase's AV work + epilogue
                for kb in range(kmax):
                    workq.append((1, make_av(po_box, pts, kb, hp, kmax, qc)))
                workq.append((1, make_epilogue(po_box, oT, hp)))
                if hp == 1:
                    workq.extend((1, u) for u in make_out_proj(qc, oT))
        for _, u in workq:
            u()

    for p in [small, work, p01, perm, const]:
        p.release()


_CACHE = {}


def _build():
    if "nc" in _CACHE:
        return _CACHE["nc"]
    nc = bacc.Bacc("TRN2", target_bir_lowering=False, debug=False, num_devices=8)
    x_d = nc.dram_tensor("x", [S, D], BF16, kind="ExternalInput").ap()
    wq_d = nc.dram_tensor("wq", [E, D], BF16, kind="ExternalInput").ap()
    wk_d = nc.dram_tensor("wk", [E, D], BF16, kind="ExternalInput").ap()
    wv_d = nc.dram_tensor("wv", [E, D], BF16, kind="ExternalInput").ap()
    wo_d = nc.dram_tensor("wo", [D, E], BF16, kind="ExternalInput").ap()
    yT_d = nc.dram_tensor("yT", [D, S], F32, kind="ExternalOutput").ap()
    with tile.TileContext(nc) as tc:
        _emit(tc, nc, x_d, wq_d, wk_d, wv_d, wo_d, yT_d)
    nc.compile()
    _CACHE["nc"] = nc
    return nc


LAST_RESULT = None


def kernel(x, wq, wk, wv, wo):
    global LAST_RESULT
    nc = _build()
    bf = ml_dtypes.bfloat16
    x = np.asarray(x, dtype=np.float32)
    wq16 = np.asarray(wq, dtype=np.float32).astype(bf)
    wk16 = np.asarray(wk, dtype=np.float32).astype(bf)
    wv16 = np.asarray(wv, dtype=np.float32).astype(bf)
    wo16 = np.asarray(wo, dtype=np.float32).astype(bf)

    in_maps = []
    for c in range(8):
        b, g = c // 4, c % 4
        rows = slice(g * E, (g + 1) * E)
        in_maps.append({
            "x": np.ascontiguousarray(x[b].astype(bf)),
            "wq": np.ascontiguousarray(wq16[rows]),
            "wk": np.ascontiguousarray(wk16[rows]),
            "wv": np.ascontiguousarray(wv16[rows]),
            "wo": np.ascontiguousarray(wo16[:, rows]),
        })

    res = bass_utils.run_bass_kernel_spmd(nc, in_maps, core_ids=list(range(8)))
    LAST_RESULT = res

    y = np.empty((B, S, D), dtype=np.float32)
    for b in range(B):
        acc = res.results[4 * b]["yT"].copy()
        for g in range(1, 4):
            acc += res.results[4 * b + g]["yT"]
        y[b] = acc.T
    return y
